# revision 12
# baseline (speedup 1.0000x reference)
"""PlainGCN message passing on 8 TRN2 NeuronCores.

Computation (reference):
    deg = bincount(h); dis = deg**-0.5; norm = dis[t]*dis[h]
    out = relu(segment_sum(norm[:,None] * x[h], t, N))

Strategy:
  - Shard edges by destination node: core c owns dest nodes
    [c*N/8, (c+1)*N/8) and all edges targeting them. x replicated.
  - norm / degree computed host-side (edge metadata, like the sharding
    hint's "shard edges (h, t, norm, ...)").
  - On device, per core: dma_gather x rows by h (int16 indices =>
    4 source buckets of 32768 rows), build one-hot(dest-in-tile)*norm
    matrices on DVE, segment-reduce via TensorE matmul accumulation in
    PSUM per 128-dest-row tile, ReLU on ScalarE, DMA out.
  - SPMD: all 8 cores share one program. Per-(tile,bucket) run lengths
    are padded to the max across cores so the static schedule is shared;
    pad edges have norm=0 (gather idx 0) and contribute nothing.
"""

import numpy as np

import concourse.bacc as bacc
import concourse.bass as bass
import concourse.mybir as mybir
import concourse.tile as tile
from concourse.bass_utils import run_bass_kernel_spmd
from concourse.library_config import mlp as mlp_lib

P = 128


def _preprocess(x, h, t, n_cores, bucket_bits, tiles_per_group):
    """Host-side edge sharding + schedule construction.

    Returns (schedule, per_core_inputs, meta) where schedule is shared by
    all cores (shapes/segment structure identical) and per_core_inputs
    holds each core's data arrays.
    """
    n, d = x.shape
    e = h.shape[0]
    assert n % n_cores == 0
    npc = n // n_cores  # nodes per core
    n_tiles = -(-npc // P)  # dest tiles per core
    bucket = 1 << bucket_bits
    n_buckets = -(-n // bucket)

    h = h.astype(np.int64)
    t = t.astype(np.int64)

    deg = np.bincount(h, minlength=n).astype(np.float32)
    # deg >= 1 guaranteed by problem setup; guard anyway (0-deg source
    # nodes never appear in h so their dis value is never used).
    dis = np.where(deg > 0, deg, 1).astype(np.float32) ** np.float32(-0.5)
    norm = (dis[t] * dis[h]).astype(np.float32)

    core = t // npc
    tloc = t - core * npc
    j = tloc // P  # dest tile within core
    tin = (tloc % P).astype(np.float32)
    b = (h >> bucket_bits).astype(np.int64)
    gidx_all = (h - (b << bucket_bits)).astype(np.int16)

    # run_len[j, b] = max over cores of per-(core,j,b) edge count, padded
    # to a multiple of 64 so every matmul segment starts at partition 0
    # or 64 (PE base-partition constraint: must be 0, 32, or 64).
    counts = np.zeros((n_cores, n_tiles, n_buckets), dtype=np.int64)
    np.add.at(counts, (core, j, b), 1)
    run_len = counts.max(axis=0)  # [n_tiles, n_buckets]
    run_len = -(-run_len // 64) * 64

    n_groups = -(-n_tiles // tiles_per_group)
    groups = [
        list(range(g * tiles_per_group, min((g + 1) * tiles_per_group, n_tiles)))
        for g in range(n_groups)
    ]

    # span lengths (shared): span (g, b) covers runs (j in groups[g], b),
    # padded to a multiple of P.
    spans = []  # (g, b, start, length) in stream coords
    seg_lists = [[] for _ in range(n_tiles)]  # per tile: (col, p0, k, b)
    pos = 0
    for g, tiles_g in enumerate(groups):
        for bb in range(n_buckets):
            s0 = pos
            for jj in tiles_g:
                r = int(run_len[jj, bb])
                # split run [pos, pos+r) at column boundaries; all pieces
                # start at partition 0 or 64 with k in {64, 128}
                q = pos
                while q < pos + r:
                    k = min(P - (q % P), pos + r - q)
                    assert q % P in (0, 64) and k in (64, P)
                    seg_lists[jj].append((q // P, q % P, k, bb))
                    q += k
                pos += r
            pos = -(-pos // P) * P  # pad span to multiple of P
            spans.append((g, bb, s0, pos - s0))
    e_pad = pos
    n_cols = e_pad // P

    # Per-core data arrays in stream order
    per_core = []
    order_key = (((core * n_groups * n_buckets) + (j // tiles_per_group) * n_buckets + b)
                 * n_tiles + j)
    sort_idx = np.argsort(order_key, kind="stable")
    cum = np.zeros((n_cores, n_tiles, n_buckets), dtype=np.int64)
    for c in range(n_cores):
        gi = np.zeros(e_pad, dtype=np.int16)
        tf = np.zeros(e_pad, dtype=np.float32)
        nf = np.zeros(e_pad, dtype=np.float32)
        sel = sort_idx[core[sort_idx] == c]
        # place this core's edges run by run into the padded stream
        # compute per-edge destination offset in stream
        jj = j[sel]
        bb2 = b[sel]
        # run start offsets in the padded stream
        run_start = np.zeros((n_tiles, n_buckets), dtype=np.int64)
        for g, tiles_g in enumerate(groups):
            for bx in range(n_buckets):
                s0 = next(s0_ for (gg, bq, s0_, _l) in spans
                          if gg == g and bq == bx)
                acc = s0
                for jx in tiles_g:
                    run_start[jx, bx] = acc
                    acc += int(run_len[jx, bx])
        # offsets within run: stable order of appearance
        within = np.zeros(len(sel), dtype=np.int64)
        cnt = {}
        key = jj * n_buckets + bb2
        # vectorized "rank within group" for sorted keys (sel is sorted by key)
        change = np.r_[True, key[1:] != key[:-1]]
        grp_id = np.cumsum(change) - 1
        first_pos = np.r_[np.nonzero(change)[0]]
        within = np.arange(len(sel)) - first_pos[grp_id]
        posn = run_start[jj, bb2] + within
        gi[posn] = gidx_all[sel]
        tf[posn] = tin[sel]
        nf[posn] = norm[sel]

        # wrap gather indices: per span, index l -> [l%16, l//16], tiled x8
        wrap = np.zeros((P, e_pad // 16), dtype=np.int16)
        for (_g, _b, s0, ln) in spans:
            w0 = s0 // 16
            seg = gi[s0:s0 + ln].reshape(ln // 16, 16).T  # [16, ln/16]
            wrap[:, w0:w0 + ln // 16] = np.tile(seg, (8, 1))

        tlocF = tf.reshape(n_cols, P).T.copy()  # [128, C]
        normF = nf.reshape(n_cols, P).T.copy()
        meta = np.concatenate([tlocF, normF], axis=1)  # [128, 2C]
        per_core.append({"gidx": wrap, "meta": meta})

    iota = np.tile(np.arange(P, dtype=np.float32), (P, 1))  # [128,128] iota[p,f]=f

    schedule = {
        "n": n, "d": d, "npc": npc, "n_tiles": n_tiles, "n_cols": n_cols,
        "e_pad": e_pad, "bucket": bucket, "n_buckets": n_buckets,
        "groups": groups, "spans": spans, "seg_lists": seg_lists,
        "run_len": run_len,
    }
    return schedule, per_core, iota


def _build_program(sched, n_cores, stage="full"):
    n, d, npc = sched["n"], sched["d"], sched["npc"]
    n_tiles, n_cols, e_pad = sched["n_tiles"], sched["n_cols"], sched["e_pad"]
    bucket, n_buckets = sched["bucket"], sched["n_buckets"]
    groups, spans, seg_lists = sched["groups"], sched["spans"], sched["seg_lists"]

    nc = bacc.Bacc("TRN2", target_bir_lowering=False, debug=False,
                   num_devices=n_cores)
    f32 = mybir.dt.float32
    x_d = nc.dram_tensor("x", [n, d], f32, kind="ExternalInput")
    iota_d = nc.dram_tensor("iota", [P, P], f32, kind="ExternalInput")
    gidx_d = nc.dram_tensor("gidx", [P, e_pad // 16], mybir.dt.int16,
                            kind="ExternalInput")
    meta_d = nc.dram_tensor("meta", [P, 2 * n_cols], f32, kind="ExternalInput")
    y_d = nc.dram_tensor("y", [npc, d], f32, kind="ExternalOutput")

    nc.gpsimd.load_library(mlp_lib)

    max_span = max(ln for (_g, _b, _s, ln) in spans)
    span_by_gb = {(g, b): (s0, ln) for (g, b, s0, ln) in spans}

    with tile.TileContext(nc) as tc:
        with (
            tc.tile_pool(name="const", bufs=1) as cpool,
            tc.tile_pool(name="gather", bufs=6) as gpool,
            tc.tile_pool(name="onehot", bufs=16) as opool,
            tc.tile_pool(name="psum", bufs=8, space="PSUM") as ppool,
            tc.tile_pool(name="outs", bufs=4) as ypool,
        ):
            iota_t = cpool.tile([P, P], f32, tag="iota")
            nc.sync.dma_start(iota_t[:], iota_d[:, :])
            meta_t = cpool.tile([P, 2 * n_cols], f32, tag="meta")
            nc.sync.dma_start(meta_t[:], meta_d[:, :])
            gidx_t = cpool.tile([P, e_pad // 16], mybir.dt.int16, tag="gidx")
            nc.sync.dma_start(gidx_t[:], gidx_d[:, :])

            for g, tiles_g in enumerate(groups):
                # gathers for this group's spans
                gtiles = {}
                for b in range(n_buckets):
                    s0, ln = span_by_gb[(g, b)]
                    if ln == 0:
                        continue
                    base = b * bucket
                    rows = min(bucket, n - base)
                    gt = gpool.tile([P, (max_span // P) * d], f32, tag="gt", name=f"gt{g}_{b}")
                    gt_3d = gt[:, :(ln // P) * d].rearrange(
                        "p (c d) -> p c d", d=d
                    )
                    nc.gpsimd.dma_gather(
                        gt_3d,
                        x_d[base:base + rows, :],
                        gidx_t[:, s0 // 16:(s0 + ln) // 16],
                        ln, ln, d,
                        single_packet=(ln <= 1024),
                    )
                    gtiles[b] = (gt, s0)

                if stage == "gather":
                    # consume gather tiles minimally: copy first column out
                    for jj in tiles_g:
                        rows = min(P, npc - jj * P)
                        yt = ypool.tile([P, d], f32, tag="yt", name=f"yt{jj}")
                        gt0, _ = gtiles[0]
                        nc.vector.tensor_copy(yt[:], gt0[:, :d])
                        nc.sync.dma_start(y_d[jj * P:jj * P + rows, :],
                                          yt[:rows, :])
                    continue

                # onehot build + matmuls; PSUM sub-groups of 4 dest tiles
                # (each tile may need 2 PSUM banks: base-0 and base-64
                # accumulation chains — PE crashes if the operand base
                # partition changes inside one accumulation group).
                oh_tiles = {}

                def build_oh(col):
                    if col not in oh_tiles:
                        oh = opool.tile([P, P], f32, tag="oh",
                                        name=f"oh{col}")
                        nc.vector.tensor_scalar(
                            oh[:], iota_t[:],
                            meta_t[:, col:col + 1],
                            meta_t[:, n_cols + col:n_cols + col + 1],
                            mybir.AluOpType.is_equal,
                            mybir.AluOpType.mult,
                        )
                        oh_tiles[col] = oh
                    return oh_tiles[col]

                if stage == "onehot":
                    for jj in tiles_g:
                        for (col, p0, k, b) in seg_lists[jj]:
                            build_oh(col)
                        rows = min(P, npc - jj * P)
                        yt = ypool.tile([P, d], f32, tag="yt",
                                        name=f"yt{jj}")
                        oh_any = next(iter(oh_tiles.values()))
                        nc.vector.tensor_copy(yt[:], oh_any[:])
                        nc.sync.dma_start(y_d[jj * P:jj * P + rows, :],
                                          yt[:rows, :])
                    continue

                for sub0 in range(0, len(tiles_g), 4):
                    for jj in tiles_g[sub0:sub0 + 4]:
                        segs = seg_lists[jj]
                        ps = {}
                        for base in (0, 64):
                            ss = [s for s in segs if s[1] == base]
                            if not ss:
                                continue
                            pt = ppool.tile([P, d], f32, tag="ps",
                                            name=f"ps{jj}_{base}")
                            ps[base] = pt
                            for si, (col, p0, k, b) in enumerate(ss):
                                oh = build_oh(col)
                                gt, s0 = gtiles[b]
                                col_l = col - s0 // P
                                nc.tensor.matmul(
                                    pt[:],
                                    lhsT=oh[p0:p0 + k, :],
                                    rhs=gt[p0:p0 + k,
                                           col_l * d:(col_l + 1) * d],
                                    start=(si == 0),
                                    stop=(si == len(ss) - 1),
                                )
                        rows = min(P, npc - jj * P)
                        yt = ypool.tile([P, d], f32, tag="yt",
                                        name=f"yt{jj}")
                        relu = mybir.ActivationFunctionType.Relu
                        if 0 in ps and 64 in ps:
                            s64 = ypool.tile([P, d], f32, tag="s64",
                                             name=f"s64_{jj}")
                            nc.scalar.activation(
                                s64[:], ps[64][:],
                                mybir.ActivationFunctionType.Identity)
                            st = ypool.tile([P, d], f32, tag="st",
                                            name=f"st{jj}")
                            nc.vector.tensor_add(st[:], s64[:], ps[0][:])
                            nc.scalar.activation(yt[:], st[:], relu)
                        elif 0 in ps:
                            nc.scalar.activation(yt[:], ps[0][:], relu)
                        elif 64 in ps:
                            nc.scalar.activation(yt[:], ps[64][:], relu)
                        else:
                            nc.vector.memset(yt[:], 0.0)
                        nc.sync.dma_start(y_d[jj * P:jj * P + rows, :],
                                          yt[:rows, :])

    nc.compile()
    return nc


def _run(x, h, t, n_cores=8, bucket_bits=15, tiles_per_group=8, trace=False):
    import time
    t0 = time.monotonic()
    sched, per_core, iota = _preprocess(x, h, t, n_cores, bucket_bits,
                                        tiles_per_group)
    t1 = time.monotonic()
    print(f"[kernel] preprocess {t1 - t0:.1f}s  e_pad={sched['e_pad']} "
          f"cols={sched['n_cols']}", flush=True)
    nc = _build_program(sched, n_cores)
    t2 = time.monotonic()
    print(f"[kernel] build+tile-schedule {t2 - t1:.1f}s", flush=True)
    in_maps = [
        {"x": np.ascontiguousarray(x), "iota": iota,
         "gidx": pc["gidx"], "meta": pc["meta"]}
        for pc in per_core
    ]
    res = run_bass_kernel_spmd(nc, in_maps, core_ids=list(range(n_cores)),
                               trace=trace)
    t3 = time.monotonic()
    print(f"[kernel] compile+run {t3 - t2:.1f}s", flush=True)
    y = np.concatenate([res.results[c]["y"] for c in range(n_cores)], axis=0)
    return y, res


def kernel(x, h, t):
    y, _ = _run(np.asarray(x), np.asarray(h), np.asarray(t))
    return y
